# revision 14
# baseline (speedup 1.0000x reference)
"""Dinov2 SDPA self-attention on one TRN2 chip (8 NeuronCores).

Problem: hidden_states [4, 1370, 1024], 16 heads x 64 dim, fp32.

Sharding (hybrid data/tensor parallel): core c handles batch b = c//2 and
head-group g = c%2 (8 heads = 512 hidden columns). Each core computes its
Q/K/V projections from its batch's activations and runs attention for its
8 heads; the host concatenates the per-core [1370, 512] context outputs.
No on-chip collectives needed.

Per-core algorithm (all layouts transposed so softmax reductions become
matmul contractions):
  Xt = X^T in SBUF  [1024, 1370]  (DMA'd as three L-slices so the first
       head pair's Q/K projection starts ~1/3 of the way into the load)
  Qt = Wq_g @ Xt + bq (per-partition bias)   [512, 1370]
  Kt = Wk_g @ Xt  (K bias is softmax-invariant -> dropped exactly)
  V  = X @ Wv_g^T + bv (natural layout, bias via DVE add; softmax weights
       sum to 1 so including bv here is exact)
  per head h: ST = Kt_h^T-tiles @ Qt_h = scores^T  [S, L] (contraction d=64;
       the head pair runs as PE row groups 0-63/64-127 writing adjacent
       PSUM banks, which overlap on HW)
  P^T = exp(ST/8) (ACT, fused 1/sqrt(d) scale; no max-subtraction -- scores
       are bounded ~|4|, exp is safe)
  ctxT_ext = [V_h | 1]^T-style stationary @ P^T = [ctx^T; rowsums]  [65, L]
  PE-transpose 128-col slices -> [L_tile, 65], normalize by column 64 via
  DVE reciprocal + per-partition tensor_scalar multiply -> out staging.

ScalarE exp is the bottleneck engine (~15M exps/core at 1 elem/lane/cycle
= ~98 us floor), and the per-engine instruction order is static, so the
emission is software-pipelined to keep ACT saturated:
  - ctx matmuls are emitted one s-tile BEHIND their scores/exp so the PE
    queue never head-of-line blocks on an exp result;
  - all non-score PE work (V projections, later head pairs' Q/K projection
    quarter-groups, previous chunk's transposes + normalize) is chopped
    into sub-us "pending units" and popped between score tiles, so no
    multi-us PE block ever sits ahead of the next scores pair.

Matmul operands are bf16 (fp32 PSUM accumulation); the unnormalized ctx
eviction stays fp32 (bf16 there would round ctx and rowsums independently
-- dominant error term). The normalized output is stored/DMA'd as bf16.
Validated vs fp32 reference: ~4.4e-3 of absmax.

All DRAM inputs are packed host-side partition-major so every input DMA is
a single instruction with long contiguous runs per partition.
"""

import os

import numpy as np
import ml_dtypes

import concourse.mybir as mybir
import concourse.tile as tile
from concourse import bacc
from concourse import bass_utils
from concourse.masks import make_identity

F32 = mybir.dt.float32
DT = mybir.dt.bfloat16
NPDT = ml_dtypes.bfloat16
AF = mybir.ActivationFunctionType

B = 4
L = 1370
HID = 1024
NH = 8            # heads per core
D = 64
QD = NH * D       # 512 projected dims per core
HP = NH // 2      # head pairs (PE row-group packing)
KC = HID // 128   # contraction chunks for projections

L_CHUNKS = [(0, 512), (512, 512), (1024, 346)]                      # moving/free dim
TILES = [(i * 128, min(128, L - i * 128)) for i in range((L + 127) // 128)]
NS = len(TILES)   # 11 (last tile 90)
# s-tiles covered by each L-slice (slice ci holds x rows l0..l0+ln)
SLICE_TILES = [(0, 4), (4, 8), (8, 11)]


def _body(nc, tc, xt_ds, wq_d, wk_d, wv_d, bq_d, bv_d, out_d):
    with tc.tile_pool(name="persist", bufs=1) as pp:
        # xt[ci]: [128, KC, ln] per L-slice (partition-major from DRAM)
        xts = [pp.tile([128, KC, ln], DT, name=f"xt{ci}")
               for ci, (l0, ln) in enumerate(L_CHUNKS)]
        wq = pp.tile([128, KC, QD], DT)
        wk = pp.tile([128, KC, QD], DT)
        wv = pp.tile([128, KC, QD], DT)
        qt = pp.tile([128, HP, L], DT)
        kt = pp.tile([128, HP, L], DT)
        vv = pp.tile([128, NS, NH, D + 1], DT)   # V tiles + ones column
        ost = pp.tile([128, NS, QD], DT)         # output staging, natural layout
        bqc = pp.tile([128, HP], F32)
        bvb = pp.tile([128, QD], F32)
        ident = pp.tile([128, 128], F32)

        make_identity(nc, ident[:, :])
        nc.vector.memset(vv[:, :, :, D:D + 1], 1.0)

        # One DMA per tensor; xt arrives L-slice by L-slice on the sync
        # queue so hp0's Q/K projections start after the first slice.
        # Weights stream concurrently on the gpsimd/scalar queues (the two
        # scalar-queue triggers retire before the first exp is ready).
        for ci in range(3):
            nc.sync.dma_start(xts[ci][:, :, :], xt_ds[ci].ap())
        nc.gpsimd.dma_start(wk[:, :, :], wk_d)
        nc.scalar.dma_start(wq[:, :, :], wq_d)
        nc.gpsimd.dma_start(wv[:, :, :], wv_d)
        nc.scalar.dma_start(bqc[:, :], bq_d)
        nc.gpsimd.dma_start(bvb[:, :], bv_d)

        # PSUM: pq 1 + stAB 2x2 + cAB 2 + tr 1 = 8 banks.  The pq/tr banks
        # double as the rotation slots for all pending projection units.
        with (
            tc.tile_pool(name="pqp", bufs=1, space="PSUM") as pqp,
            tc.tile_pool(name="sps", bufs=2, space="PSUM") as sps,
            tc.tile_pool(name="cps", bufs=1, space="PSUM") as cps,
            tc.tile_pool(name="tps", bufs=1, space="PSUM") as tps,
            tc.tile_pool(name="wp", bufs=3) as wp,
        ):
            PR = [(pqp, "pq"), (tps, "tr")]
            pr_i = 0

            def pr_slot():
                nonlocal pr_i
                s = PR[pr_i % 2]
                pr_i += 1
                return s

            # ---- pending-unit machinery ----------------------------------
            # Each unit is (deadline, closure) emitting <= ~1 us of PE work
            # (plus its DVE eviction).  Units are popped FIFO between score
            # tiles; before each scores pair every unit whose deadline
            # (g, si) has arrived is force-drained, which guarantees all
            # producers are emitted ahead of their consumers.
            pending = []

            def drain_due(g, si):
                while any(d <= (g, si) for d, _ in pending):
                    pending.pop(0)[1]()

            def pop_front(n):
                for _ in range(min(n, len(pending))):
                    pending.pop(0)[1]()

            def v_units(si):
                s0, ss = TILES[si]
                ci = next(c for c, (a, b) in enumerate(SLICE_TILES)
                          if a <= si < b)
                o = s0 - L_CHUNKS[ci][0]
                st = {}

                def half(h, st=st, si=si, ci=ci, o=o, ss=ss):
                    if h == 0:
                        pl, tg = pr_slot()
                        st["t"] = pl.tile([128, QD], F32, name="vps", tag=tg)
                        ks = range(0, KC // 2)
                    else:
                        ks = range(KC // 2, KC)
                    for k in ks:
                        nc.tensor.matmul(st["t"][:ss, :], xts[ci][:, k, o:o + ss],
                                         wv[:, k, :],
                                         start=(k == 0), stop=(k == KC - 1))
                    if h == 1:
                        nc.vector.tensor_add(
                            vv[:ss, si, :, 0:D],
                            st["t"][:ss, :].rearrange("p (h d) -> p h d", h=NH),
                            bvb[:ss, :].rearrange("p (h d) -> p h d", h=NH),
                        )
                return [((0, si), lambda: half(0)),
                        ((0, si), lambda: half(1))]

            def qk_units(hp, ci):
                m = slice(hp * 128, (hp + 1) * 128)
                l0, ln = L_CHUNKS[ci]
                st = {}

                def quarter(which, h, st=st, m=m, ci=ci, l0=l0, ln=ln, hp=hp):
                    if h == 0:
                        pl, tg = pr_slot()
                        st[which] = pl.tile([128, 512], F32,
                                            name=which + "ps", tag=tg)
                        ks = range(0, KC // 2)
                    else:
                        ks = range(KC // 2, KC)
                    w_ = wq if which == "q" else wk
                    for k in ks:
                        nc.tensor.matmul(st[which][:, :ln], w_[:, k, m],
                                         xts[ci][:, k, :],
                                         start=(k == 0), stop=(k == KC - 1))
                    if h == 1 and which == "q":
                        nc.vector.tensor_scalar_add(
                            qt[:, hp, l0:l0 + ln], st["q"][:, :ln],
                            bqc[:, hp:hp + 1])
                    elif h == 1:
                        nc.vector.tensor_copy(kt[:, hp, l0:l0 + ln],
                                              st["k"][:, :ln])
                # q feeds scores of chunk (hp, ci); k covers the s-range of
                # slice ci, read from (hp, c0)'s s-tiles onward
                qd = (hp * 3 + ci, -1)
                kd = (hp * 3, SLICE_TILES[ci][0] - 1)
                return [(qd, lambda: quarter("q", 0)),
                        (qd, lambda: quarter("q", 1)),
                        (kd, lambda: quarter("k", 0)),
                        (kd, lambda: quarter("k", 1))]

            def norm_units(hp, l0, ln, ctAB, last):
                hA, hB = 2 * hp, 2 * hp + 1
                units = []
                for j in range(0, ln, 128):
                    for h2, h in ((0, hA), (1, hB)):
                        def tn(j=j, h2=h2, h=h, l0=l0, ln=ln, ctAB=ctAB,
                               last=last):
                            lt = (l0 + j) // 128
                            w = min(128, ln - j)
                            if last:
                                # final chunk: every attention bank is free
                                pl, tg = ((tps, "tr"), (pqp, "pq"),
                                          (cps, "cAB"), (sps, "stAB"))[
                                    ((j // 128) * 2 + h2) % 4]
                                tr = pl.tile([128, 65], F32, name="trx", tag=tg)
                            else:
                                pl, tg = pr_slot()
                                tr = pl.tile([128, 65], F32, name="tr", tag=tg)
                            nc.tensor.transpose(tr[:w, :], ctAB[:, h2, j:j + w],
                                                ident[0:65, 0:65])
                            rc = wp.tile([128, 1], F32, name="rc", tag="rc")
                            nc.vector.reciprocal(rc[:w, :], tr[:w, 64:65])
                            nc.vector.tensor_scalar_mul(
                                ost[:w, lt, h * D:(h + 1) * D],
                                tr[:w, 0:D], rc[:w, :])
                        units.append(((12, 0), tn))
                return units

            # ---- ramp ----------------------------------------------------
            # hp0 chunk-0 Q/K projection first (unblocks the first scores),
            # V tile 0 next (unblocks the first ctx); everything else joins
            # the pending queue in deadline order, consumed inside the
            # attention windows.
            for _, u in qk_units(0, 0):
                u()
            for _, u in v_units(0):
                u()
            for si in range(1, NS):
                pending.extend(v_units(si))
            pending.extend(qk_units(0, 1))
            pending.extend(qk_units(0, 2))
            pending.sort(key=lambda du: du[0])

            # ---- software-pipelined attention ----------------------------
            prev = None   # (hp, l0, ln, cAB) awaiting evict + normalize
            for hp in range(HP):
                hA, hB = 2 * hp, 2 * hp + 1
                for ci, (l0, ln) in enumerate(L_CHUNKS):
                    g = hp * 3 + ci
                    cAB = cps.tile([65, 2, 512], F32, name="cAB", tag="cAB")
                    eprev = None
                    for si, (s0, ss) in enumerate(TILES):
                        drain_due(g, si)
                        stAB = sps.tile([128, 2, 512], F32, name="stAB",
                                        tag="stAB")
                        nc.tensor.matmul(stAB[:ss, 0, :ln],
                                         kt[0:64, hp, s0:s0 + ss],
                                         qt[0:64, hp, l0:l0 + ln],
                                         start=True, stop=True,
                                         tile_position=(0, 0))
                        nc.tensor.matmul(stAB[:ss, 1, :ln],
                                         kt[64:128, hp, s0:s0 + ss],
                                         qt[64:128, hp, l0:l0 + ln],
                                         start=True, stop=True,
                                         tile_position=(64, 0))
                        eAB = wp.tile([128, 2, 512], DT, name="eAB", tag="eAB")
                        nc.scalar.activation(eAB[:ss, :, :ln],
                                             stAB[:ss, :, :ln],
                                             AF.Exp, scale=0.125)
                        if si == 0 and prev is not None:
                            # evict the previous chunk's accumulator, then
                            # queue its transposes/normalize
                            phq, pl0, pln, pcAB = prev
                            ctAB = wp.tile([65, 2, 512], F32, name="ctAB",
                                           tag="ctAB")
                            nc.vector.tensor_copy(ctAB[:, :, :pln],
                                                  pcAB[:, :, :pln])
                            pending.extend(
                                norm_units(phq, pl0, pln, ctAB, last=False))
                        # spread pending PE work between score tiles;
                        # drain faster when the queue is long (ramp)
                        q = len(pending)
                        pop_front(3 if q > 20 else 2 if q > 10 else 1)
                        if eprev is not None:
                            psi, pss, peAB = eprev
                            nc.tensor.matmul(cAB[:, 0, :ln],
                                             vv[:pss, psi, hA, :],
                                             peAB[:pss, 0, :ln],
                                             start=(psi == 0), stop=False)
                            nc.tensor.matmul(cAB[:, 1, :ln],
                                             vv[:pss, psi, hB, :],
                                             peAB[:pss, 1, :ln],
                                             start=(psi == 0), stop=False)
                        eprev = (si, ss, eAB)
                    psi, pss, peAB = eprev
                    nc.tensor.matmul(cAB[:, 0, :ln], vv[:pss, psi, hA, :],
                                     peAB[:pss, 0, :ln],
                                     start=False, stop=True)
                    nc.tensor.matmul(cAB[:, 1, :ln], vv[:pss, psi, hB, :],
                                     peAB[:pss, 1, :ln],
                                     start=False, stop=True)
                    if hp + 1 < HP:
                        pending.extend(qk_units(hp + 1, ci))
                    prev = (hp, l0, ln, cAB)

            # ---- tail ----------------------------------------------------
            pop_front(len(pending))
            phq, pl0, pln, pcAB = prev
            ctAB = wp.tile([65, 2, 512], F32, name="ctAB", tag="ctAB")
            nc.vector.tensor_copy(ctAB[:, :, :pln], pcAB[:, :, :pln])
            for _, u in norm_units(phq, pl0, pln, ctAB, last=True):
                u()

            # two output DMAs: tiles 0-9 form a regular partition-major
            # block; the ragged last tile (90 rows) goes separately
            nc.sync.dma_start(
                out_d[0:(NS - 1) * 128, :].rearrange("(t p) n -> p t n", p=128),
                ost[:, 0:NS - 1, :])
            t0, tn = TILES[NS - 1]
            nc.sync.dma_start(out_d[t0:t0 + tn, :], ost[:tn, NS - 1, :])


_NC_CACHE = {}


def _build(reps=1):
    key = ("nc", reps)
    if key in _NC_CACHE:
        return _NC_CACHE[key]
    nc = bacc.Bacc("TRN2", target_bir_lowering=False, debug=False)
    xt_ds = [nc.dram_tensor(f"xt{ci}", [128, KC, ln], DT, kind="ExternalInput")
             for ci, (l0, ln) in enumerate(L_CHUNKS)]
    wq_d = nc.dram_tensor("wqt", [128, KC, QD], DT, kind="ExternalInput")
    wk_d = nc.dram_tensor("wkt", [128, KC, QD], DT, kind="ExternalInput")
    wv_d = nc.dram_tensor("wvt", [128, KC, QD], DT, kind="ExternalInput")
    bq_d = nc.dram_tensor("bq", [128, HP], F32, kind="ExternalInput")
    bv_d = nc.dram_tensor("bvb", [128, QD], F32, kind="ExternalInput")
    out_d = nc.dram_tensor("out", [L, QD], DT, kind="ExternalOutput")

    with tile.TileContext(nc) as tc:
        for _ in range(reps):
            _body(nc, tc, xt_ds, wq_d.ap(), wk_d.ap(), wv_d.ap(),
                  bq_d.ap(), bv_d.ap(), out_d.ap())
    nc.compile()
    return nc


def make_in_maps(hidden_states, Wq, bq, Wk, bk, Wv, bv):
    hs = np.asarray(hidden_states)
    Wq, bq, Wk, Wv, bv = map(np.asarray, (Wq, bq, Wk, Wv, bv))
    in_maps = []
    for c in range(8):
        b, g = divmod(c, 2)
        gs = slice(g * QD, (g + 1) * QD)
        xt = np.ascontiguousarray(hs[b].T).astype(NPDT)      # [1024, 1370]
        xt = xt.reshape(KC, 128, L)                          # [k, p, l]
        m = {}
        for ci, (l0, ln) in enumerate(L_CHUNKS):
            m[f"xt{ci}"] = np.ascontiguousarray(
                xt[:, :, l0:l0 + ln].transpose(1, 0, 2))     # [p, k, ln]
        for nm, W in (("wqt", Wq), ("wkt", Wk), ("wvt", Wv)):
            wt = np.ascontiguousarray(W[gs, :].T).astype(NPDT)   # [1024, 512]
            m[nm] = np.ascontiguousarray(
                wt.reshape(KC, 128, QD).transpose(1, 0, 2))      # [p, k, n]
        m["bq"] = np.ascontiguousarray(
            bq[gs].reshape(HP, 128).T).astype(np.float32)        # [p, hp]
        m["bvb"] = np.ascontiguousarray(
            np.broadcast_to(bv[gs], (128, QD))).astype(np.float32)
        in_maps.append(m)
    return in_maps


LAST_RESULTS = None


def kernel(hidden_states, Wq, bq, Wk, bk, Wv, bv):
    global LAST_RESULTS
    nc = _build()
    in_maps = make_in_maps(hidden_states, Wq, bq, Wk, bk, Wv, bv)
    try:
        res = bass_utils.run_bass_kernel_spmd(
            nc, in_maps, core_ids=list(range(8)),
            trace=bool(os.environ.get("KERNEL_TRACE")),
        )
    except (ImportError, ModuleNotFoundError):
        # The axon NTFF profiling hook is absent in some containers; retry
        # with tracing disabled rather than failing the run.
        prev = os.environ.get("BASS_NEVER_TRACE")
        os.environ["BASS_NEVER_TRACE"] = "1"
        try:
            res = bass_utils.run_bass_kernel_spmd(
                nc, in_maps, core_ids=list(range(8)))
        finally:
            if prev is None:
                os.environ.pop("BASS_NEVER_TRACE", None)
            else:
                os.environ["BASS_NEVER_TRACE"] = prev
    LAST_RESULTS = res
    out = np.empty((B, L, HID), np.float32)
    for c, om in enumerate(res.results):
        b, g = divmod(c, 2)
        out[b, :, g * QD:(g + 1) * QD] = om["out"].astype(np.float32)
    return out


# revision 15
# speedup vs baseline: 1.2970x; 1.2970x over previous
"""Dinov2 SDPA self-attention on one TRN2 chip (8 NeuronCores).

Problem: hidden_states [4, 1370, 1024], 16 heads x 64 dim, fp32.

Sharding (hybrid data/tensor parallel): core c handles batch b = c//2 and
head-group g = c%2 (8 heads = 512 hidden columns). Each core computes its
Q/K/V projections from its batch's activations and runs attention for its
8 heads; the host concatenates the per-core [1370, 512] context outputs.
No on-chip collectives needed.

Per-core algorithm (all layouts transposed so softmax reductions become
matmul contractions):
  Xt = X^T in SBUF  [1024, 1370]  (DMA'd as three L-slices so the first
       head pair's Q/K projection starts ~1/3 of the way into the load)
  Qt = Wq_g @ Xt + bq (per-partition bias)   [512, 1370]
  Kt = Wk_g @ Xt  (K bias is softmax-invariant -> dropped exactly)
  V  = X @ Wv_g^T + bv (natural layout, bias via DVE add; softmax weights
       sum to 1 so including bv here is exact)
  per head h: ST = Kt_h^T-tiles @ Qt_h = scores^T  [S, L] (contraction d=64;
       the head pair runs as PE row groups 0-63/64-127 writing adjacent
       PSUM banks, which overlap on HW)
  P^T = exp(ST/8) (ACT, fused 1/sqrt(d) scale; no max-subtraction -- scores
       are bounded ~|4|, exp is safe)
  ctxT_ext = [V_h | 1]^T-style stationary @ P^T = [ctx^T; rowsums]  [65, L]
  PE-transpose 128-col slices -> [L_tile, 65], normalize by column 64 via
  DVE reciprocal + per-partition tensor_scalar multiply -> out staging.

ScalarE exp is the bottleneck engine (~15M exps/core at 1 elem/lane/cycle
= ~98 us floor), and the per-engine instruction order is static, so the
emission is software-pipelined to keep ACT saturated:
  - ctx matmuls are emitted one s-tile BEHIND their scores/exp so the PE
    queue never head-of-line blocks on an exp result;
  - all non-score PE work (V projections, later head pairs' Q/K projection
    quarter-groups, previous chunk's transposes + normalize) is chopped
    into sub-us "pending units" and popped between score tiles, so no
    multi-us PE block ever sits ahead of the next scores pair.

Matmul operands are bf16 (fp32 PSUM accumulation); the unnormalized ctx
eviction stays fp32 (bf16 there would round ctx and rowsums independently
-- dominant error term). The normalized output is stored/DMA'd as bf16.
Validated vs fp32 reference: ~4.4e-3 of absmax.

All DRAM inputs are packed host-side partition-major so every input DMA is
a single instruction with long contiguous runs per partition.
"""

import os

import numpy as np
import ml_dtypes

import concourse.mybir as mybir
import concourse.tile as tile
from concourse import bacc
from concourse import bass_utils
from concourse.masks import make_identity

F32 = mybir.dt.float32
DT = mybir.dt.bfloat16
NPDT = ml_dtypes.bfloat16
AF = mybir.ActivationFunctionType

B = 4
L = 1370
HID = 1024
NH = 8            # heads per core
D = 64
QD = NH * D       # 512 projected dims per core
HP = NH // 2      # head pairs (PE row-group packing)
KC = HID // 128   # contraction chunks for projections

L_CHUNKS = [(0, 512), (512, 512), (1024, 346)]                      # moving/free dim
TILES = [(i * 128, min(128, L - i * 128)) for i in range((L + 127) // 128)]
NS = len(TILES)   # 11 (last tile 90)
# s-tiles covered by each L-slice (slice ci holds x rows l0..l0+ln)
SLICE_TILES = [(0, 4), (4, 8), (8, 11)]


def _body(nc, tc, xt_ds, wq_d, wk_d, wv_d, bq_d, bv_d, out_d):
    with tc.tile_pool(name="persist", bufs=1) as pp:
        # xt[ci]: [128, KC, ln] per L-slice (partition-major from DRAM)
        xts = [pp.tile([128, KC, ln], DT, name=f"xt{ci}")
               for ci, (l0, ln) in enumerate(L_CHUNKS)]
        wq = pp.tile([128, KC, QD], DT)
        wk = pp.tile([128, KC, QD], DT)
        wv = pp.tile([128, KC, QD], DT)
        qt = pp.tile([128, HP, L], DT)
        kt = pp.tile([128, HP, L], DT)
        vv = pp.tile([128, NS, NH, D + 1], DT)   # V tiles + ones column
        ost = pp.tile([128, NS, QD], DT)         # output staging, natural layout
        bqc = pp.tile([128, HP], F32)
        bvb = pp.tile([128, QD], F32)
        ident = pp.tile([128, 128], F32)

        make_identity(nc, ident[:, :])
        nc.vector.memset(vv[:, :, :, D:D + 1], 1.0)

        # One DMA per tensor; xt arrives L-slice by L-slice on the sync
        # queue so hp0's Q/K projections start after the first slice.
        # Weights stream concurrently on the gpsimd/scalar queues (the two
        # scalar-queue triggers retire before the first exp is ready).
        for ci in range(3):
            nc.sync.dma_start(xts[ci][:, :, :], xt_ds[ci].ap())
        nc.gpsimd.dma_start(wk[:, :, :], wk_d)
        nc.scalar.dma_start(wq[:, :, :], wq_d)
        nc.gpsimd.dma_start(wv[:, :, :], wv_d)
        nc.scalar.dma_start(bqc[:, :], bq_d)
        nc.gpsimd.dma_start(bvb[:, :], bv_d)

        # PSUM: pq 1 + stAB 2x2 + cAB 2 + tr 1 = 8 banks.  The pq/tr banks
        # double as the rotation slots for all pending projection units.
        with (
            tc.tile_pool(name="pqp", bufs=1, space="PSUM") as pqp,
            tc.tile_pool(name="sps", bufs=2, space="PSUM") as sps,
            tc.tile_pool(name="cps", bufs=1, space="PSUM") as cps,
            tc.tile_pool(name="tps", bufs=1, space="PSUM") as tps,
            tc.tile_pool(name="wp", bufs=3) as wp,
        ):
            PR = [(pqp, "pq"), (tps, "tr")]
            pr_i = 0

            def pr_slot():
                nonlocal pr_i
                s = PR[pr_i % 2]
                pr_i += 1
                return s

            # ---- pending-unit machinery ----------------------------------
            # Each unit is (deadline, closure) emitting <= ~1 us of PE work
            # (plus its DVE eviction).  Units are popped FIFO between score
            # tiles; before each scores pair every unit whose deadline
            # (g, si) has arrived is force-drained, which guarantees all
            # producers are emitted ahead of their consumers.
            pending = []

            def drain_due(g, si):
                while any(d <= (g, si) for d, _ in pending):
                    pending.pop(0)[1]()

            def pop_front(n):
                for _ in range(min(n, len(pending))):
                    pending.pop(0)[1]()

            def v_units(si):
                s0, ss = TILES[si]
                ci = next(c for c, (a, b) in enumerate(SLICE_TILES)
                          if a <= si < b)
                o = s0 - L_CHUNKS[ci][0]
                st = {}

                def half(h, st=st, si=si, ci=ci, o=o, ss=ss):
                    if h == 0:
                        pl, tg = pr_slot()
                        st["t"] = pl.tile([128, QD], F32, name="vps", tag=tg)
                        ks = range(0, KC // 2)
                    else:
                        ks = range(KC // 2, KC)
                    for k in ks:
                        nc.tensor.matmul(st["t"][:ss, :], xts[ci][:, k, o:o + ss],
                                         wv[:, k, :],
                                         start=(k == 0), stop=(k == KC - 1))
                    if h == 1:
                        nc.vector.tensor_add(
                            vv[:ss, si, :, 0:D],
                            st["t"][:ss, :].rearrange("p (h d) -> p h d", h=NH),
                            bvb[:ss, :].rearrange("p (h d) -> p h d", h=NH),
                        )
                return [((0, si), lambda: half(0)),
                        ((0, si), lambda: half(1))]

            def qk_units(hp, ci):
                m = slice(hp * 128, (hp + 1) * 128)
                l0, ln = L_CHUNKS[ci]
                st = {}

                def quarter(which, h, st=st, m=m, ci=ci, l0=l0, ln=ln, hp=hp):
                    if h == 0:
                        pl, tg = pr_slot()
                        st[which] = pl.tile([128, 512], F32,
                                            name=which + "ps", tag=tg)
                        ks = range(0, KC // 2)
                    else:
                        ks = range(KC // 2, KC)
                    w_ = wq if which == "q" else wk
                    for k in ks:
                        nc.tensor.matmul(st[which][:, :ln], w_[:, k, m],
                                         xts[ci][:, k, :],
                                         start=(k == 0), stop=(k == KC - 1))
                    if h == 1 and which == "q":
                        nc.vector.tensor_scalar_add(
                            qt[:, hp, l0:l0 + ln], st["q"][:, :ln],
                            bqc[:, hp:hp + 1])
                    elif h == 1:
                        nc.vector.tensor_copy(kt[:, hp, l0:l0 + ln],
                                              st["k"][:, :ln])
                # q feeds scores of chunk (hp, ci); k covers the s-range of
                # slice ci, read from (hp, c0)'s s-tiles onward
                qd = (hp * 3 + ci, -1)
                kd = (hp * 3, SLICE_TILES[ci][0] - 1)
                return [(qd, lambda: quarter("q", 0)),
                        (qd, lambda: quarter("q", 1)),
                        (kd, lambda: quarter("k", 0)),
                        (kd, lambda: quarter("k", 1))]

            def norm_units(hp, l0, ln, ctAB, last):
                hA, hB = 2 * hp, 2 * hp + 1
                units = []
                for j in range(0, ln, 128):
                    for h2, h in ((0, hA), (1, hB)):
                        def tn(j=j, h2=h2, h=h, l0=l0, ln=ln, ctAB=ctAB,
                               last=last):
                            lt = (l0 + j) // 128
                            w = min(128, ln - j)
                            if last:
                                # final chunk: every attention bank is free
                                pl, tg = ((tps, "tr"), (pqp, "pq"),
                                          (cps, "cAB"), (sps, "stAB"))[
                                    ((j // 128) * 2 + h2) % 4]
                                tr = pl.tile([128, 65], F32, name="trx", tag=tg)
                            else:
                                pl, tg = pr_slot()
                                tr = pl.tile([128, 65], F32, name="tr", tag=tg)
                            nc.tensor.transpose(tr[:w, :], ctAB[:, h2, j:j + w],
                                                ident[0:65, 0:65])
                            rc = wp.tile([128, 1], F32, name="rc", tag="rc")
                            nc.vector.reciprocal(rc[:w, :], tr[:w, 64:65])
                            nc.vector.tensor_scalar_mul(
                                ost[:w, lt, h * D:(h + 1) * D],
                                tr[:w, 0:D], rc[:w, :])
                        units.append(((12, 0), tn))
                return units

            # ---- ramp ----------------------------------------------------
            # hp0 chunk-0 Q/K projection first (unblocks the first scores),
            # V tile 0 next (unblocks the first ctx); everything else joins
            # the pending queue in deadline order, consumed inside the
            # attention windows.
            for _, u in qk_units(0, 0):
                u()
            for _, u in v_units(0):
                u()
            for si in range(1, NS):
                pending.extend(v_units(si))
            pending.extend(qk_units(0, 1))
            pending.extend(qk_units(0, 2))
            pending.sort(key=lambda du: du[0])

            # ---- software-pipelined attention ----------------------------
            prev = None   # (hp, l0, ln, cAB) awaiting evict + normalize
            for hp in range(HP):
                hA, hB = 2 * hp, 2 * hp + 1
                for ci, (l0, ln) in enumerate(L_CHUNKS):
                    g = hp * 3 + ci
                    cAB = cps.tile([65, 2, 512], F32, name="cAB", tag="cAB")
                    eprev = None
                    for si, (s0, ss) in enumerate(TILES):
                        drain_due(g, si)
                        stAB = sps.tile([128, 2, 512], F32, name="stAB",
                                        tag="stAB")
                        nc.tensor.matmul(stAB[:ss, 0, :ln],
                                         kt[0:64, hp, s0:s0 + ss],
                                         qt[0:64, hp, l0:l0 + ln],
                                         start=True, stop=True,
                                         tile_position=(0, 0))
                        nc.tensor.matmul(stAB[:ss, 1, :ln],
                                         kt[64:128, hp, s0:s0 + ss],
                                         qt[64:128, hp, l0:l0 + ln],
                                         start=True, stop=True,
                                         tile_position=(64, 0))
                        eAB = wp.tile([128, 2, 512], DT, name="eAB", tag="eAB")
                        nc.scalar.activation(eAB[:ss, :, :ln],
                                             stAB[:ss, :, :ln],
                                             AF.Exp, scale=0.125)
                        if si == 0 and prev is not None:
                            # evict the previous chunk's accumulator, then
                            # queue its transposes/normalize
                            phq, pl0, pln, pcAB = prev
                            ctAB = wp.tile([65, 2, 512], F32, name="ctAB",
                                           tag="ctAB")
                            nc.vector.tensor_copy(ctAB[:, :, :pln],
                                                  pcAB[:, :, :pln])
                            pending.extend(
                                norm_units(phq, pl0, pln, ctAB, last=False))
                        # spread pending PE work between score tiles;
                        # drain faster when the queue is long (ramp)
                        q = len(pending)
                        pop_front(3 if q > 20 else 2 if q > 10 else 1)
                        if eprev is not None:
                            psi, pss, peAB = eprev
                            nc.tensor.matmul(cAB[:, 0, :ln],
                                             vv[:pss, psi, hA, :],
                                             peAB[:pss, 0, :ln],
                                             start=(psi == 0), stop=False)
                            nc.tensor.matmul(cAB[:, 1, :ln],
                                             vv[:pss, psi, hB, :],
                                             peAB[:pss, 1, :ln],
                                             start=(psi == 0), stop=False)
                        eprev = (si, ss, eAB)
                    psi, pss, peAB = eprev
                    nc.tensor.matmul(cAB[:, 0, :ln], vv[:pss, psi, hA, :],
                                     peAB[:pss, 0, :ln],
                                     start=False, stop=True)
                    nc.tensor.matmul(cAB[:, 1, :ln], vv[:pss, psi, hB, :],
                                     peAB[:pss, 1, :ln],
                                     start=False, stop=True)
                    if hp + 1 < HP:
                        pending.extend(qk_units(hp + 1, ci))
                    prev = (hp, l0, ln, cAB)

            # ---- tail ----------------------------------------------------
            pop_front(len(pending))
            phq, pl0, pln, pcAB = prev
            ctAB = wp.tile([65, 2, 512], F32, name="ctAB", tag="ctAB")
            nc.vector.tensor_copy(ctAB[:, :, :pln], pcAB[:, :, :pln])
            for _, u in norm_units(phq, pl0, pln, ctAB, last=True):
                u()

            # per-tile output DMAs so early tiles stream out as soon as the
            # last head pair's normalize finishes them
            for ti, (t0, tn) in enumerate(TILES):
                nc.sync.dma_start(out_d[t0:t0 + tn, :], ost[:tn, ti, :])


_NC_CACHE = {}


def _build(reps=1):
    key = ("nc", reps)
    if key in _NC_CACHE:
        return _NC_CACHE[key]
    nc = bacc.Bacc("TRN2", target_bir_lowering=False, debug=False)
    xt_ds = [nc.dram_tensor(f"xt{ci}", [128, KC, ln], DT, kind="ExternalInput")
             for ci, (l0, ln) in enumerate(L_CHUNKS)]
    wq_d = nc.dram_tensor("wqt", [128, KC, QD], DT, kind="ExternalInput")
    wk_d = nc.dram_tensor("wkt", [128, KC, QD], DT, kind="ExternalInput")
    wv_d = nc.dram_tensor("wvt", [128, KC, QD], DT, kind="ExternalInput")
    bq_d = nc.dram_tensor("bq", [128, HP], F32, kind="ExternalInput")
    bv_d = nc.dram_tensor("bvb", [128, QD], F32, kind="ExternalInput")
    out_d = nc.dram_tensor("out", [L, QD], DT, kind="ExternalOutput")

    with tile.TileContext(nc) as tc:
        for _ in range(reps):
            _body(nc, tc, xt_ds, wq_d.ap(), wk_d.ap(), wv_d.ap(),
                  bq_d.ap(), bv_d.ap(), out_d.ap())
    nc.compile()
    return nc


def make_in_maps(hidden_states, Wq, bq, Wk, bk, Wv, bv):
    hs = np.asarray(hidden_states)
    Wq, bq, Wk, Wv, bv = map(np.asarray, (Wq, bq, Wk, Wv, bv))
    in_maps = []
    for c in range(8):
        b, g = divmod(c, 2)
        gs = slice(g * QD, (g + 1) * QD)
        xt = np.ascontiguousarray(hs[b].T).astype(NPDT)      # [1024, 1370]
        xt = xt.reshape(KC, 128, L)                          # [k, p, l]
        m = {}
        for ci, (l0, ln) in enumerate(L_CHUNKS):
            m[f"xt{ci}"] = np.ascontiguousarray(
                xt[:, :, l0:l0 + ln].transpose(1, 0, 2))     # [p, k, ln]
        for nm, W in (("wqt", Wq), ("wkt", Wk), ("wvt", Wv)):
            wt = np.ascontiguousarray(W[gs, :].T).astype(NPDT)   # [1024, 512]
            m[nm] = np.ascontiguousarray(
                wt.reshape(KC, 128, QD).transpose(1, 0, 2))      # [p, k, n]
        m["bq"] = np.ascontiguousarray(
            bq[gs].reshape(HP, 128).T).astype(np.float32)        # [p, hp]
        m["bvb"] = np.ascontiguousarray(
            np.broadcast_to(bv[gs], (128, QD))).astype(np.float32)
        in_maps.append(m)
    return in_maps


LAST_RESULTS = None


def kernel(hidden_states, Wq, bq, Wk, bk, Wv, bv):
    global LAST_RESULTS
    nc = _build()
    in_maps = make_in_maps(hidden_states, Wq, bq, Wk, bk, Wv, bv)
    try:
        res = bass_utils.run_bass_kernel_spmd(
            nc, in_maps, core_ids=list(range(8)),
            trace=bool(os.environ.get("KERNEL_TRACE")),
        )
    except (ImportError, ModuleNotFoundError):
        # The axon NTFF profiling hook is absent in some containers; retry
        # with tracing disabled rather than failing the run.
        prev = os.environ.get("BASS_NEVER_TRACE")
        os.environ["BASS_NEVER_TRACE"] = "1"
        try:
            res = bass_utils.run_bass_kernel_spmd(
                nc, in_maps, core_ids=list(range(8)))
        finally:
            if prev is None:
                os.environ.pop("BASS_NEVER_TRACE", None)
            else:
                os.environ["BASS_NEVER_TRACE"] = prev
    LAST_RESULTS = res
    out = np.empty((B, L, HID), np.float32)
    for c, om in enumerate(res.results):
        b, g = divmod(c, 2)
        out[b, :, g * QD:(g + 1) * QD] = om["out"].astype(np.float32)
    return out


# revision 17
# speedup vs baseline: 1.7599x; 1.3569x over previous
"""Dinov2 SDPA self-attention on one TRN2 chip (8 NeuronCores).

Problem: hidden_states [4, 1370, 1024], 16 heads x 64 dim, fp32.

Sharding (hybrid data/tensor parallel): core c handles batch b = c//2 and
head-group g = c%2 (8 heads = 512 hidden columns). Each core computes its
Q/K/V projections from its batch's activations and runs attention for its
8 heads; the host concatenates the per-core [1370, 512] context outputs.
No on-chip collectives needed.

Per-core algorithm (all layouts transposed so softmax reductions become
matmul contractions):
  Xt = X^T in SBUF  [1024, 1370]  (DMA'd as three L-slices so the first
       head pair's Q/K projection starts ~1/3 of the way into the load)
  Qt = Wq_g @ Xt + bq (per-partition bias)   [512, 1370]
  Kt = Wk_g @ Xt  (K bias is softmax-invariant -> dropped exactly)
  V  = X @ Wv_g^T + bv (natural layout, bias via DVE add; softmax weights
       sum to 1 so including bv here is exact)
  per head h: ST = Kt_h^T-tiles @ Qt_h = scores^T  [S, L] (contraction d=64;
       the head pair runs as PE row groups 0-63/64-127 writing adjacent
       PSUM banks, which overlap on HW)
  P^T = exp(ST/8) (ACT, fused 1/sqrt(d) scale; no max-subtraction -- scores
       are bounded ~|4|, exp is safe)
  ctxT_ext = [V_h | 1]^T-style stationary @ P^T = [ctx^T; rowsums]  [65, L]
  PE-transpose 128-col slices -> [L_tile, 65], normalize by column 64 via
  DVE reciprocal + per-partition tensor_scalar multiply -> out staging.

ScalarE exp is the bottleneck engine (~15M exps/core at 1 elem/lane/cycle
= ~98 us floor), and the per-engine instruction order is static, so the
emission is software-pipelined to keep ACT saturated:
  - ctx matmuls are emitted one s-tile BEHIND their scores/exp so the PE
    queue never head-of-line blocks on an exp result;
  - all non-score PE work (V projections, later head pairs' Q/K projection
    quarter-groups, previous chunk's transposes + normalize) is chopped
    into sub-us "pending units" and popped between score tiles, so no
    multi-us PE block ever sits ahead of the next scores pair.

Matmul operands are bf16 (fp32 PSUM accumulation); the unnormalized ctx
eviction stays fp32 (bf16 there would round ctx and rowsums independently
-- dominant error term). The normalized output is stored/DMA'd as bf16.
Validated vs fp32 reference: ~4.4e-3 of absmax.

All DRAM inputs are packed host-side partition-major so every input DMA is
a single instruction with long contiguous runs per partition.
"""

import os

import numpy as np
import ml_dtypes

import concourse.mybir as mybir
import concourse.tile as tile
from concourse import bacc
from concourse import bass_utils
from concourse.masks import make_identity

F32 = mybir.dt.float32
DT = mybir.dt.bfloat16
NPDT = ml_dtypes.bfloat16
AF = mybir.ActivationFunctionType

B = 4
L = 1370
HID = 1024
NH = 8            # heads per core
D = 64
QD = NH * D       # 512 projected dims per core
HP = NH // 2      # head pairs (PE row-group packing)
KC = HID // 128   # contraction chunks for projections

L_CHUNKS = [(0, 512), (512, 512), (1024, 346)]                      # moving/free dim
TILES = [(i * 128, min(128, L - i * 128)) for i in range((L + 127) // 128)]
NS = len(TILES)   # 11 (last tile 90)
# s-tiles covered by each L-slice (slice ci holds x rows l0..l0+ln)
SLICE_TILES = [(0, 4), (4, 8), (8, 11)]


def _body(nc, tc, xt_ds, wq_d, wk_d, wv_d, bq_d, bv_d, out_d):
    with tc.tile_pool(name="persist", bufs=1) as pp:
        # xt[ci]: [128, KC, ln] per L-slice (partition-major from DRAM)
        xts = [pp.tile([128, KC, ln], DT, name=f"xt{ci}")
               for ci, (l0, ln) in enumerate(L_CHUNKS)]
        wq = pp.tile([128, KC, QD], DT)
        wk = pp.tile([128, KC, QD], DT)
        wv = pp.tile([128, KC, QD], DT)
        qt = pp.tile([128, HP, L], DT)
        kt = pp.tile([128, HP, L], DT)
        vv = pp.tile([128, NS, NH, D + 1], DT)   # V tiles + ones column
        ost = pp.tile([128, NS, QD], DT)         # output staging, natural layout
        bqc = pp.tile([128, HP], F32)
        bvb = pp.tile([128, QD], F32)
        ident = pp.tile([128, 128], F32)

        make_identity(nc, ident[:, :])
        nc.vector.memset(vv[:, :, :, D:D + 1], 1.0)

        # One DMA per tensor; xt arrives L-slice by L-slice on the sync
        # queue so hp0's Q/K projections start after the first slice.
        # Weights stream concurrently on the gpsimd/scalar queues (the two
        # scalar-queue triggers retire before the first exp is ready).
        for ci in range(3):
            nc.sync.dma_start(xts[ci][:, :, :], xt_ds[ci].ap())
        nc.gpsimd.dma_start(wk[:, :, :], wk_d)
        nc.scalar.dma_start(wq[:, :, :], wq_d)
        nc.gpsimd.dma_start(wv[:, :, :], wv_d)
        nc.scalar.dma_start(bqc[:, :], bq_d)
        nc.gpsimd.dma_start(bvb[:, :], bv_d)

        # PSUM: pq 1 + stAB 2x2 + cAB 2 + tr 1 = 8 banks.  The pq/tr banks
        # double as the rotation slots for all pending projection units.
        with (
            tc.tile_pool(name="pqp", bufs=1, space="PSUM") as pqp,
            tc.tile_pool(name="sps", bufs=2, space="PSUM") as sps,
            tc.tile_pool(name="cps", bufs=1, space="PSUM") as cps,
            tc.tile_pool(name="tps", bufs=1, space="PSUM") as tps,
            tc.tile_pool(name="wp", bufs=3) as wp,
        ):
            PR = [(pqp, "pq"), (tps, "tr")]
            pr_i = 0

            def pr_slot():
                nonlocal pr_i
                s = PR[pr_i % 2]
                pr_i += 1
                return s

            # ---- pending-unit machinery ----------------------------------
            # Each unit is (deadline, closure) emitting <= ~1 us of PE work
            # (plus its DVE eviction).  Units are popped FIFO between score
            # tiles; before each scores pair every unit whose deadline
            # (g, si) has arrived is force-drained, which guarantees all
            # producers are emitted ahead of their consumers.
            pending = []

            def drain_due(g, si):
                while any(d <= (g, si) for d, _ in pending):
                    pending.pop(0)[1]()

            def pop_front(n):
                for _ in range(min(n, len(pending))):
                    pending.pop(0)[1]()

            def v_units(si):
                s0, ss = TILES[si]
                ci = next(c for c, (a, b) in enumerate(SLICE_TILES)
                          if a <= si < b)
                o = s0 - L_CHUNKS[ci][0]
                st = {}

                def half(h, st=st, si=si, ci=ci, o=o, ss=ss):
                    if h == 0:
                        pl, tg = pr_slot()
                        st["t"] = pl.tile([128, QD], F32, name="vps", tag=tg)
                        ks = range(0, KC // 2)
                    else:
                        ks = range(KC // 2, KC)
                    for k in ks:
                        nc.tensor.matmul(st["t"][:ss, :], xts[ci][:, k, o:o + ss],
                                         wv[:, k, :],
                                         start=(k == 0), stop=(k == KC - 1))
                    if h == 1:
                        nc.vector.tensor_add(
                            vv[:ss, si, :, 0:D],
                            st["t"][:ss, :].rearrange("p (h d) -> p h d", h=NH),
                            bvb[:ss, :].rearrange("p (h d) -> p h d", h=NH),
                        )
                return [((0, si), lambda: half(0)),
                        ((0, si), lambda: half(1))]

            def qk_units(hp, ci):
                m = slice(hp * 128, (hp + 1) * 128)
                l0, ln = L_CHUNKS[ci]
                st = {}

                def quarter(which, h, st=st, m=m, ci=ci, l0=l0, ln=ln, hp=hp):
                    if h == 0:
                        pl, tg = pr_slot()
                        st[which] = pl.tile([128, 512], F32,
                                            name=which + "ps", tag=tg)
                        ks = range(0, KC // 2)
                    else:
                        ks = range(KC // 2, KC)
                    w_ = wq if which == "q" else wk
                    for k in ks:
                        nc.tensor.matmul(st[which][:, :ln], w_[:, k, m],
                                         xts[ci][:, k, :],
                                         start=(k == 0), stop=(k == KC - 1))
                    if h == 1 and which == "q":
                        nc.vector.tensor_scalar_add(
                            qt[:, hp, l0:l0 + ln], st["q"][:, :ln],
                            bqc[:, hp:hp + 1])
                    elif h == 1:
                        nc.vector.tensor_copy(kt[:, hp, l0:l0 + ln],
                                              st["k"][:, :ln])
                # q feeds scores of chunk (hp, ci); k covers the s-range of
                # slice ci, read from (hp, c0)'s s-tiles onward
                qd = (hp * 3 + ci, -1)
                kd = (hp * 3, SLICE_TILES[ci][0] - 1)
                return [(qd, lambda: quarter("q", 0)),
                        (qd, lambda: quarter("q", 1)),
                        (kd, lambda: quarter("k", 0)),
                        (kd, lambda: quarter("k", 1))]

            def norm_units(hp, l0, ln, ctAB, last):
                hA, hB = 2 * hp, 2 * hp + 1
                units = []
                for j in range(0, ln, 128):
                    for h2, h in ((0, hA), (1, hB)):
                        def tn(j=j, h2=h2, h=h, l0=l0, ln=ln, ctAB=ctAB,
                               last=last):
                            lt = (l0 + j) // 128
                            w = min(128, ln - j)
                            if last:
                                # final chunk: every attention bank is free
                                pl, tg = ((tps, "tr"), (pqp, "pq"),
                                          (cps, "cAB"), (sps, "stAB"))[
                                    ((j // 128) * 2 + h2) % 4]
                                tr = pl.tile([128, 65], F32, name="trx", tag=tg)
                            else:
                                pl, tg = pr_slot()
                                tr = pl.tile([128, 65], F32, name="tr", tag=tg)
                            nc.tensor.transpose(tr[:w, :], ctAB[:, h2, j:j + w],
                                                ident[0:65, 0:65])
                            rc = wp.tile([128, 1], F32, name="rc", tag="rc")
                            nc.vector.reciprocal(rc[:w, :], tr[:w, 64:65])
                            nc.vector.tensor_scalar_mul(
                                ost[:w, lt, h * D:(h + 1) * D],
                                tr[:w, 0:D], rc[:w, :])
                        units.append(((12, 0), tn))
                return units

            # ---- ramp ----------------------------------------------------
            # hp0 chunk-0 Q/K projection first (unblocks the first scores),
            # V tile 0 next (unblocks the first ctx); everything else joins
            # the pending queue in deadline order, consumed inside the
            # attention windows.
            for _, u in qk_units(0, 0):
                u()
            for _, u in v_units(0):
                u()
            for si in range(1, NS):
                pending.extend(v_units(si))
            pending.extend(qk_units(0, 1))
            pending.extend(qk_units(0, 2))
            pending.sort(key=lambda du: du[0])

            # ---- software-pipelined attention ----------------------------
            prev = None   # (hp, l0, ln, cAB) awaiting evict + normalize
            for hp in range(HP):
                hA, hB = 2 * hp, 2 * hp + 1
                for ci, (l0, ln) in enumerate(L_CHUNKS):
                    g = hp * 3 + ci
                    cAB = cps.tile([65, 2, 512], F32, name="cAB", tag="cAB")
                    eprev = None
                    for si, (s0, ss) in enumerate(TILES):
                        drain_due(g, si)
                        stAB = sps.tile([128, 2, 512], F32, name="stAB",
                                        tag="stAB")
                        nc.tensor.matmul(stAB[:ss, 0, :ln],
                                         kt[0:64, hp, s0:s0 + ss],
                                         qt[0:64, hp, l0:l0 + ln],
                                         start=True, stop=True,
                                         tile_position=(0, 0))
                        nc.tensor.matmul(stAB[:ss, 1, :ln],
                                         kt[64:128, hp, s0:s0 + ss],
                                         qt[64:128, hp, l0:l0 + ln],
                                         start=True, stop=True,
                                         tile_position=(64, 0))
                        eAB = wp.tile([128, 2, 512], DT, name="eAB", tag="eAB")
                        nc.scalar.activation(eAB[:ss, :, :ln],
                                             stAB[:ss, :, :ln],
                                             AF.Exp, scale=0.125)
                        if si == 0 and prev is not None:
                            # evict the previous chunk's accumulator, then
                            # queue its transposes/normalize
                            phq, pl0, pln, pcAB = prev
                            ctAB = wp.tile([65, 2, 512], F32, name="ctAB",
                                           tag="ctAB")
                            nc.vector.tensor_copy(ctAB[:, :, :pln],
                                                  pcAB[:, :, :pln])
                            pending.extend(
                                norm_units(phq, pl0, pln, ctAB, last=False))
                        # spread pending PE work between score tiles;
                        # drain faster when the queue is long (ramp)
                        q = len(pending)
                        pop_front(3 if q > 20 else 2 if q > 10 else 1)
                        if eprev is not None:
                            psi, pss, peAB = eprev
                            nc.tensor.matmul(cAB[:, 0, :ln],
                                             vv[:pss, psi, hA, :],
                                             peAB[:pss, 0, :ln],
                                             start=(psi == 0), stop=False)
                            nc.tensor.matmul(cAB[:, 1, :ln],
                                             vv[:pss, psi, hB, :],
                                             peAB[:pss, 1, :ln],
                                             start=(psi == 0), stop=False)
                        eprev = (si, ss, eAB)
                    psi, pss, peAB = eprev
                    nc.tensor.matmul(cAB[:, 0, :ln], vv[:pss, psi, hA, :],
                                     peAB[:pss, 0, :ln],
                                     start=False, stop=True)
                    nc.tensor.matmul(cAB[:, 1, :ln], vv[:pss, psi, hB, :],
                                     peAB[:pss, 1, :ln],
                                     start=False, stop=True)
                    if hp + 1 < HP:
                        pending.extend(qk_units(hp + 1, ci))
                    prev = (hp, l0, ln, cAB)

            # ---- tail ----------------------------------------------------
            pop_front(len(pending))
            phq, pl0, pln, pcAB = prev
            ctAB = wp.tile([65, 2, 512], F32, name="ctAB", tag="ctAB")
            nc.vector.tensor_copy(ctAB[:, :, :pln], pcAB[:, :, :pln])
            for _, u in norm_units(phq, pl0, pln, ctAB, last=True):
                u()

            for ti, (t0, tn) in enumerate(TILES):
                nc.sync.dma_start(out_d[t0:t0 + tn, :], ost[:tn, ti, :])


_NC_CACHE = {}


def _build(reps=1):
    key = ("nc", reps)
    if key in _NC_CACHE:
        return _NC_CACHE[key]
    nc = bacc.Bacc("TRN2", target_bir_lowering=False, debug=False)
    xt_ds = [nc.dram_tensor(f"xt{ci}", [128, KC, ln], DT, kind="ExternalInput")
             for ci, (l0, ln) in enumerate(L_CHUNKS)]
    wq_d = nc.dram_tensor("wqt", [128, KC, QD], DT, kind="ExternalInput")
    wk_d = nc.dram_tensor("wkt", [128, KC, QD], DT, kind="ExternalInput")
    wv_d = nc.dram_tensor("wvt", [128, KC, QD], DT, kind="ExternalInput")
    bq_d = nc.dram_tensor("bq", [128, HP], F32, kind="ExternalInput")
    bv_d = nc.dram_tensor("bvb", [128, QD], F32, kind="ExternalInput")
    out_d = nc.dram_tensor("out", [L, QD], DT, kind="ExternalOutput")

    with tile.TileContext(nc) as tc:
        for _ in range(reps):
            _body(nc, tc, xt_ds, wq_d.ap(), wk_d.ap(), wv_d.ap(),
                  bq_d.ap(), bv_d.ap(), out_d.ap())
    nc.compile()
    return nc


def make_in_maps(hidden_states, Wq, bq, Wk, bk, Wv, bv):
    hs = np.asarray(hidden_states)
    Wq, bq, Wk, Wv, bv = map(np.asarray, (Wq, bq, Wk, Wv, bv))
    in_maps = []
    for c in range(8):
        b, g = divmod(c, 2)
        gs = slice(g * QD, (g + 1) * QD)
        xt = np.ascontiguousarray(hs[b].T).astype(NPDT)      # [1024, 1370]
        xt = xt.reshape(KC, 128, L)                          # [k, p, l]
        m = {}
        for ci, (l0, ln) in enumerate(L_CHUNKS):
            m[f"xt{ci}"] = np.ascontiguousarray(
                xt[:, :, l0:l0 + ln].transpose(1, 0, 2))     # [p, k, ln]
        for nm, W in (("wqt", Wq), ("wkt", Wk), ("wvt", Wv)):
            wt = np.ascontiguousarray(W[gs, :].T).astype(NPDT)   # [1024, 512]
            m[nm] = np.ascontiguousarray(
                wt.reshape(KC, 128, QD).transpose(1, 0, 2))      # [p, k, n]
        m["bq"] = np.ascontiguousarray(
            bq[gs].reshape(HP, 128).T).astype(np.float32)        # [p, hp]
        m["bvb"] = np.ascontiguousarray(
            np.broadcast_to(bv[gs], (128, QD))).astype(np.float32)
        in_maps.append(m)
    return in_maps


LAST_RESULTS = None


def kernel(hidden_states, Wq, bq, Wk, bk, Wv, bv):
    global LAST_RESULTS
    nc = _build()
    in_maps = make_in_maps(hidden_states, Wq, bq, Wk, bk, Wv, bv)
    try:
        res = bass_utils.run_bass_kernel_spmd(
            nc, in_maps, core_ids=list(range(8)),
            trace=bool(os.environ.get("KERNEL_TRACE")),
        )
    except (ImportError, ModuleNotFoundError):
        # The axon NTFF profiling hook is absent in some containers; retry
        # with tracing disabled rather than failing the run.
        prev = os.environ.get("BASS_NEVER_TRACE")
        os.environ["BASS_NEVER_TRACE"] = "1"
        try:
            res = bass_utils.run_bass_kernel_spmd(
                nc, in_maps, core_ids=list(range(8)))
        finally:
            if prev is None:
                os.environ.pop("BASS_NEVER_TRACE", None)
            else:
                os.environ["BASS_NEVER_TRACE"] = prev
    LAST_RESULTS = res
    out = np.empty((B, L, HID), np.float32)
    for c, om in enumerate(res.results):
        b, g = divmod(c, 2)
        out[b, :, g * QD:(g + 1) * QD] = om["out"].astype(np.float32)
    return out


# revision 19
# speedup vs baseline: 1.7654x; 1.0031x over previous
"""Dinov2 SDPA self-attention on one TRN2 chip (8 NeuronCores).

Problem: hidden_states [4, 1370, 1024], 16 heads x 64 dim, fp32.

Sharding (hybrid data/tensor parallel): core c handles batch b = c//2 and
head-group g = c%2 (8 heads = 512 hidden columns). Each core computes its
Q/K/V projections from its batch's activations and runs attention for its
8 heads; the host concatenates the per-core [1370, 512] context outputs.
No on-chip collectives needed.

Per-core algorithm (all layouts transposed so softmax reductions become
matmul contractions):
  Xt = X^T in SBUF  [1024, 1370]  (DMA'd as three L-slices so the first
       head pair's Q/K projection starts ~1/3 of the way into the load)
  Qt = Wq_g @ Xt + bq (per-partition bias)   [512, 1370]
  Kt = Wk_g @ Xt  (K bias is softmax-invariant -> dropped exactly)
  V  = X @ Wv_g^T + bv (natural layout, bias via DVE add; softmax weights
       sum to 1 so including bv here is exact)
  per head h: ST = Kt_h^T-tiles @ Qt_h = scores^T  [S, L] (contraction d=64;
       the head pair runs as PE row groups 0-63/64-127 writing adjacent
       PSUM banks, which overlap on HW)
  P^T = exp(ST/8) (ACT, fused 1/sqrt(d) scale; no max-subtraction -- scores
       are bounded ~|4|, exp is safe)
  ctxT_ext = [V_h | 1]^T-style stationary @ P^T = [ctx^T; rowsums]  [65, L]
  PE-transpose 128-col slices -> [L_tile, 65], normalize by column 64 via
  DVE reciprocal + per-partition tensor_scalar multiply -> out staging.

ScalarE exp is the bottleneck engine (~15M exps/core at 1 elem/lane/cycle
= ~98 us floor), and the per-engine instruction order is static, so the
emission is software-pipelined to keep ACT saturated:
  - ctx matmuls are emitted one s-tile BEHIND their scores/exp so the PE
    queue never head-of-line blocks on an exp result;
  - all non-score PE work (V projections, later head pairs' Q/K projection
    quarter-groups, previous chunk's transposes + normalize) is chopped
    into sub-us "pending units" and popped between score tiles, so no
    multi-us PE block ever sits ahead of the next scores pair.

Matmul operands are bf16 (fp32 PSUM accumulation); the unnormalized ctx
eviction stays fp32 (bf16 there would round ctx and rowsums independently
-- dominant error term). The normalized output is stored/DMA'd as bf16.
Validated vs fp32 reference: ~4.4e-3 of absmax.

All DRAM inputs are packed host-side partition-major so every input DMA is
a single instruction with long contiguous runs per partition.
"""

import os

import numpy as np
import ml_dtypes

import concourse.mybir as mybir
import concourse.tile as tile
from concourse import bacc
from concourse import bass_utils
from concourse.masks import make_identity

F32 = mybir.dt.float32
DT = mybir.dt.bfloat16
NPDT = ml_dtypes.bfloat16
AF = mybir.ActivationFunctionType

B = 4
L = 1370
HID = 1024
NH = 8            # heads per core
D = 64
QD = NH * D       # 512 projected dims per core
HP = NH // 2      # head pairs (PE row-group packing)
KC = HID // 128   # contraction chunks for projections

L_CHUNKS = [(0, 512), (512, 512), (1024, 346)]                      # moving/free dim
TILES = [(i * 128, min(128, L - i * 128)) for i in range((L + 127) // 128)]
NS = len(TILES)   # 11 (last tile 90)
# s-tiles covered by each L-slice (slice ci holds x rows l0..l0+ln)
SLICE_TILES = [(0, 4), (4, 8), (8, 11)]


def _body(nc, tc, xt_ds, wq_d, wk_d, wv_d, bq_d, bv_d, out_d):
    with tc.tile_pool(name="persist", bufs=1) as pp:
        # xt[ci]: [128, KC, ln] per L-slice (partition-major from DRAM)
        xts = [pp.tile([128, KC, ln], DT, name=f"xt{ci}")
               for ci, (l0, ln) in enumerate(L_CHUNKS)]
        wq = pp.tile([128, KC, QD], DT)
        wk = pp.tile([128, KC, QD], DT)
        wv = pp.tile([128, KC, QD], DT)
        qt = pp.tile([128, HP, L], DT)
        kt = pp.tile([128, HP, L], DT)
        vv = pp.tile([128, NS, NH, D + 1], DT)   # V tiles + ones column
        ost = pp.tile([128, NS, QD], DT)         # output staging, natural layout
        bqc = pp.tile([128, HP], F32)
        bvb = pp.tile([128, QD], F32)
        ident = pp.tile([128, 128], F32)

        make_identity(nc, ident[:, :])
        nc.vector.memset(vv[:, :, :, D:D + 1], 1.0)

        # One DMA per tensor; xt arrives L-slice by L-slice on the sync
        # queue so hp0's Q/K projections start after the first slice.
        # Weights stream concurrently on the gpsimd/scalar queues (the two
        # scalar-queue triggers retire before the first exp is ready).
        for ci in range(3):
            nc.sync.dma_start(xts[ci][:, :, :], xt_ds[ci].ap())
        nc.gpsimd.dma_start(wk[:, :, :], wk_d)
        nc.scalar.dma_start(wq[:, :, :], wq_d)
        nc.gpsimd.dma_start(wv[:, :, :], wv_d)
        nc.scalar.dma_start(bqc[:, :], bq_d)
        nc.gpsimd.dma_start(bvb[:, :], bv_d)

        # PSUM: pq 1 + stAB 2x2 + cAB 2 + tr 1 = 8 banks.  The pq/tr banks
        # double as the rotation slots for all pending projection units.
        with (
            tc.tile_pool(name="pqp", bufs=1, space="PSUM") as pqp,
            tc.tile_pool(name="sps", bufs=2, space="PSUM") as sps,
            tc.tile_pool(name="cps", bufs=1, space="PSUM") as cps,
            tc.tile_pool(name="tps", bufs=1, space="PSUM") as tps,
            tc.tile_pool(name="wp", bufs=3) as wp,
        ):
            PR = [(pqp, "pq"), (tps, "tr")]
            pr_i = 0

            def pr_slot():
                nonlocal pr_i
                s = PR[pr_i % 2]
                pr_i += 1
                return s

            # ---- pending-unit machinery ----------------------------------
            # Each unit is (deadline, closure) emitting <= ~1 us of PE work
            # (plus its DVE eviction).  Units are popped FIFO between score
            # tiles; before each scores pair every unit whose deadline
            # (g, si) has arrived is force-drained, which guarantees all
            # producers are emitted ahead of their consumers.
            pending = []

            def drain_due(g, si):
                while any(d <= (g, si) for d, _ in pending):
                    pending.pop(0)[1]()

            def pop_front(n):
                for _ in range(min(n, len(pending))):
                    pending.pop(0)[1]()

            def v_units(si):
                s0, ss = TILES[si]
                ci = next(c for c, (a, b) in enumerate(SLICE_TILES)
                          if a <= si < b)
                o = s0 - L_CHUNKS[ci][0]
                st = {}

                def half(h, st=st, si=si, ci=ci, o=o, ss=ss):
                    if h == 0:
                        pl, tg = pr_slot()
                        st["t"] = pl.tile([128, QD], F32, name="vps", tag=tg)
                        ks = range(0, KC // 2)
                    else:
                        ks = range(KC // 2, KC)
                    for k in ks:
                        nc.tensor.matmul(st["t"][:ss, :], xts[ci][:, k, o:o + ss],
                                         wv[:, k, :],
                                         start=(k == 0), stop=(k == KC - 1))
                    if h == 1:
                        nc.vector.tensor_add(
                            vv[:ss, si, :, 0:D],
                            st["t"][:ss, :].rearrange("p (h d) -> p h d", h=NH),
                            bvb[:ss, :].rearrange("p (h d) -> p h d", h=NH),
                        )
                return [((0, si), lambda: half(0)),
                        ((0, si), lambda: half(1))]

            def qk_units(hp, ci):
                m = slice(hp * 128, (hp + 1) * 128)
                l0, ln = L_CHUNKS[ci]
                st = {}

                def quarter(which, h, st=st, m=m, ci=ci, l0=l0, ln=ln, hp=hp):
                    if h == 0:
                        pl, tg = pr_slot()
                        st[which] = pl.tile([128, 512], F32,
                                            name=which + "ps", tag=tg)
                        ks = range(0, KC // 2)
                    else:
                        ks = range(KC // 2, KC)
                    w_ = wq if which == "q" else wk
                    for k in ks:
                        nc.tensor.matmul(st[which][:, :ln], w_[:, k, m],
                                         xts[ci][:, k, :],
                                         start=(k == 0), stop=(k == KC - 1))
                    if h == 1 and which == "q":
                        nc.vector.tensor_scalar_add(
                            qt[:, hp, l0:l0 + ln], st["q"][:, :ln],
                            bqc[:, hp:hp + 1])
                    elif h == 1:
                        nc.vector.tensor_copy(kt[:, hp, l0:l0 + ln],
                                              st["k"][:, :ln])
                # q feeds scores of chunk (hp, ci); k covers the s-range of
                # slice ci, read from (hp, c0)'s s-tiles onward
                qd = (hp * 3 + ci, -1)
                kd = (hp * 3, SLICE_TILES[ci][0] - 1)
                return [(qd, lambda: quarter("q", 0)),
                        (qd, lambda: quarter("q", 1)),
                        (kd, lambda: quarter("k", 0)),
                        (kd, lambda: quarter("k", 1))]

            def norm_units(hp, l0, ln, ctAB, last):
                hA, hB = 2 * hp, 2 * hp + 1
                units = []
                for j in range(0, ln, 128):
                    for h2, h in ((0, hA), (1, hB)):
                        def tn(j=j, h2=h2, h=h, l0=l0, ln=ln, ctAB=ctAB,
                               last=last):
                            lt = (l0 + j) // 128
                            w = min(128, ln - j)
                            if last:
                                # final chunk: every attention bank is free
                                pl, tg = ((tps, "tr"), (pqp, "pq"),
                                          (cps, "cAB"), (sps, "stAB"))[
                                    ((j // 128) * 2 + h2) % 4]
                                tr = pl.tile([128, 65], F32, name="trx", tag=tg)
                            else:
                                pl, tg = pr_slot()
                                tr = pl.tile([128, 65], F32, name="tr", tag=tg)
                            nc.tensor.transpose(tr[:w, :], ctAB[:, h2, j:j + w],
                                                ident[0:65, 0:65])
                            rc = wp.tile([128, 1], F32, name="rc", tag="rc")
                            nc.vector.reciprocal(rc[:w, :], tr[:w, 64:65])
                            nc.vector.tensor_scalar_mul(
                                ost[:w, lt, h * D:(h + 1) * D],
                                tr[:w, 0:D], rc[:w, :])
                        units.append(((12, 0), tn))
                return units

            # ---- ramp ----------------------------------------------------
            # hp0 chunk-0 Q/K projection first (unblocks the first scores),
            # V tile 0 next (unblocks the first ctx); everything else joins
            # the pending queue in deadline order, consumed inside the
            # attention windows.
            for _, u in qk_units(0, 0):
                u()
            for _, u in v_units(0):
                u()
            for si in range(1, NS):
                pending.extend(v_units(si))
            pending.extend(qk_units(0, 1))
            pending.extend(qk_units(0, 2))
            pending.sort(key=lambda du: du[0])

            # ---- software-pipelined attention ----------------------------
            prev = None   # (hp, l0, ln, cAB) awaiting evict + normalize
            for hp in range(HP):
                hA, hB = 2 * hp, 2 * hp + 1
                for ci, (l0, ln) in enumerate(L_CHUNKS):
                    g = hp * 3 + ci
                    cAB = cps.tile([65, 2, 512], F32, name="cAB", tag="cAB")
                    eprev = None
                    for si, (s0, ss) in enumerate(TILES):
                        drain_due(g, si)
                        stAB = sps.tile([128, 2, 512], F32, name="stAB",
                                        tag="stAB")
                        nc.tensor.matmul(stAB[:ss, 0, :ln],
                                         kt[0:64, hp, s0:s0 + ss],
                                         qt[0:64, hp, l0:l0 + ln],
                                         start=True, stop=True,
                                         tile_position=(0, 0))
                        nc.tensor.matmul(stAB[:ss, 1, :ln],
                                         kt[64:128, hp, s0:s0 + ss],
                                         qt[64:128, hp, l0:l0 + ln],
                                         start=True, stop=True,
                                         tile_position=(64, 0))
                        eAB = wp.tile([128, 2, 512], DT, name="eAB", tag="eAB")
                        nc.scalar.activation(eAB[:ss, :, :ln],
                                             stAB[:ss, :, :ln],
                                             AF.Exp, scale=0.125)
                        if si == 0 and prev is not None:
                            # evict the previous chunk's accumulator, then
                            # queue its transposes/normalize
                            phq, pl0, pln, pcAB = prev
                            ctAB = wp.tile([65, 2, 512], F32, name="ctAB",
                                           tag="ctAB")
                            nc.vector.tensor_copy(ctAB[:, :, :pln],
                                                  pcAB[:, :, :pln])
                            pending.extend(
                                norm_units(phq, pl0, pln, ctAB, last=False))
                        # spread pending PE work between score tiles;
                        # drain faster when the queue is long (ramp)
                        q = len(pending)
                        pop_front(3 if q > 20 else 2 if q > 10 else 1)
                        if eprev is not None:
                            psi, pss, peAB = eprev
                            nc.tensor.matmul(cAB[:, 0, :ln],
                                             vv[:pss, psi, hA, :],
                                             peAB[:pss, 0, :ln],
                                             start=(psi == 0), stop=False)
                            nc.tensor.matmul(cAB[:, 1, :ln],
                                             vv[:pss, psi, hB, :],
                                             peAB[:pss, 1, :ln],
                                             start=(psi == 0), stop=False)
                        eprev = (si, ss, eAB)
                    psi, pss, peAB = eprev
                    nc.tensor.matmul(cAB[:, 0, :ln], vv[:pss, psi, hA, :],
                                     peAB[:pss, 0, :ln],
                                     start=False, stop=True)
                    nc.tensor.matmul(cAB[:, 1, :ln], vv[:pss, psi, hB, :],
                                     peAB[:pss, 1, :ln],
                                     start=False, stop=True)
                    if hp + 1 < HP:
                        pending.extend(qk_units(hp + 1, ci))
                    prev = (hp, l0, ln, cAB)

            # ---- tail ----------------------------------------------------
            pop_front(len(pending))
            phq, pl0, pln, pcAB = prev
            ctAB = wp.tile([65, 2, 512], F32, name="ctAB", tag="ctAB")
            nc.vector.tensor_copy(ctAB[:, :, :pln], pcAB[:, :, :pln])
            for _, u in norm_units(phq, pl0, pln, ctAB, last=True):
                u()

            for ti, (t0, tn) in enumerate(TILES):
                nc.sync.dma_start(out_d[t0:t0 + tn, :], ost[:tn, ti, :])


_NC_CACHE = {}


def _build(reps=1):
    key = ("nc", reps)
    if key in _NC_CACHE:
        return _NC_CACHE[key]
    nc = bacc.Bacc("TRN2", target_bir_lowering=False, debug=False)
    xt_ds = [nc.dram_tensor(f"xt{ci}", [128, KC, ln], DT, kind="ExternalInput")
             for ci, (l0, ln) in enumerate(L_CHUNKS)]
    wq_d = nc.dram_tensor("wqt", [128, KC, QD], DT, kind="ExternalInput")
    wk_d = nc.dram_tensor("wkt", [128, KC, QD], DT, kind="ExternalInput")
    wv_d = nc.dram_tensor("wvt", [128, KC, QD], DT, kind="ExternalInput")
    bq_d = nc.dram_tensor("bq", [128, HP], F32, kind="ExternalInput")
    bv_d = nc.dram_tensor("bvb", [128, QD], F32, kind="ExternalInput")
    out_d = nc.dram_tensor("out", [L, QD], DT, kind="ExternalOutput")

    with tile.TileContext(nc) as tc:
        for _ in range(reps):
            _body(nc, tc, xt_ds, wq_d.ap(), wk_d.ap(), wv_d.ap(),
                  bq_d.ap(), bv_d.ap(), out_d.ap())
    nc.compile()
    return nc


def make_in_maps(hidden_states, Wq, bq, Wk, bk, Wv, bv):
    hs = np.asarray(hidden_states)
    Wq, bq, Wk, Wv, bv = map(np.asarray, (Wq, bq, Wk, Wv, bv))
    in_maps = []
    for c in range(8):
        b, g = divmod(c, 2)
        gs = slice(g * QD, (g + 1) * QD)
        xt = np.ascontiguousarray(hs[b].T).astype(NPDT)      # [1024, 1370]
        xt = xt.reshape(KC, 128, L)                          # [k, p, l]
        m = {}
        for ci, (l0, ln) in enumerate(L_CHUNKS):
            m[f"xt{ci}"] = np.ascontiguousarray(
                xt[:, :, l0:l0 + ln].transpose(1, 0, 2))     # [p, k, ln]
        for nm, W in (("wqt", Wq), ("wkt", Wk), ("wvt", Wv)):
            wt = np.ascontiguousarray(W[gs, :].T).astype(NPDT)   # [1024, 512]
            m[nm] = np.ascontiguousarray(
                wt.reshape(KC, 128, QD).transpose(1, 0, 2))      # [p, k, n]
        m["bq"] = np.ascontiguousarray(
            bq[gs].reshape(HP, 128).T).astype(np.float32)        # [p, hp]
        m["bvb"] = np.ascontiguousarray(
            np.broadcast_to(bv[gs], (128, QD))).astype(np.float32)
        in_maps.append(m)
    return in_maps


LAST_RESULTS = None


def kernel(hidden_states, Wq, bq, Wk, bk, Wv, bv):
    global LAST_RESULTS
    nc = _build()
    in_maps = make_in_maps(hidden_states, Wq, bq, Wk, bk, Wv, bv)
    try:
        res = bass_utils.run_bass_kernel_spmd(
            nc, in_maps, core_ids=list(range(8)),
            trace=bool(os.environ.get("KERNEL_TRACE")),
        )
    except (ImportError, ModuleNotFoundError):
        # The axon NTFF profiling hook is absent in some containers; retry
        # with tracing disabled rather than failing the run.
        prev = os.environ.get("BASS_NEVER_TRACE")
        os.environ["BASS_NEVER_TRACE"] = "1"
        try:
            res = bass_utils.run_bass_kernel_spmd(
                nc, in_maps, core_ids=list(range(8)))
        finally:
            if prev is None:
                os.environ.pop("BASS_NEVER_TRACE", None)
            else:
                os.environ["BASS_NEVER_TRACE"] = prev
    LAST_RESULTS = res
    out = np.empty((B, L, HID), np.float32)
    for c, om in enumerate(res.results):
        b, g = divmod(c, 2)
        out[b, :, g * QD:(g + 1) * QD] = om["out"].astype(np.float32)
    return out
